# revision 3
# baseline (speedup 1.0000x reference)
"""SAGAN-style self-attention block on 8 trn2 NeuronCores.

Full inputs: x [8, 512, 64, 64], w_theta [64, 512], w_phi [64, 512],
w_g [256, 512], w_o [512, 256], gamma scalar.

Sharding: data-parallel over batch — one batch item per core. Each core runs
an identical Bass program over its own x[b]; weights are replicated.

Per-core math (C=512, n=H*W=4096, m=n/4=1024):
  theta = w_theta @ x            [64, 4096]
  phi   = pool2(w_phi @ x)       [64, 1024]
  g     = pool2(w_g @ x)         [256, 1024]
  S^T   = phi^T @ theta          [1024, 4096]   (scores, transposed layout)
  E     = exp(S^T)               (no max-subtraction needed: |S| < ~50)
  Z     = ones^T @ E             [*, 4096]      (row sums, broadcast layout)
  att   = (g @ E) / Z            [256, 4096]
  out   = (gamma*w_o) @ att + x  [512, 4096]

All matmuls run as float32r (full-rate fp32 on the PE at N>=512; tf32-like
input rounding, ~1.5e-4 rel err). The residual add uses unrounded fp32 x.
"""

from contextlib import ExitStack

import numpy as np

import bass_rust
import concourse.bass as bass
import concourse.mybir as mybir
import concourse.tile as tile
from concourse.bass_utils import run_bass_kernel_spmd
from concourse.masks import make_identity

P = 128
C = 512  # channels
C8 = 64  # theta/phi channels
C2 = 256  # g channels
N = 4096  # H*W
M = 1024  # pooled spatial
NS = 8  # n-slices
SL = 512  # n-slice width
MT = 8  # m-tiles of 128
F32 = mybir.dt.float32
F32R = mybir.dt.float32r
AX = mybir.AxisListType
ALU = mybir.AluOpType
ACTF = mybir.ActivationFunctionType


def _pool_view(ap):
    """[p, 512] slice of the conv output -> 5D maxpool view [p, h2, w2, dy, dx].

    Within an n-slice of 512 = 8 image rows: local n = (2*h2+dy)*64 + 2*w2+dx.
    """
    return ap.rearrange("p (h2 dy w2 dx) -> p h2 w2 dy dx", h2=4, dy=2, w2=32, dx=2)


def emit(nc, tc, ctx):
    x_f = nc.dram_tensor("x", [C, N], F32, kind="ExternalInput")
    x_r = nc.dram_tensor("x2", [C, N], F32R, kind="ExternalInput")
    wproj = nc.dram_tensor("wproj", [C, 384], F32R, kind="ExternalInput")
    wo = nc.dram_tensor("wo", [C2, C], F32R, kind="ExternalInput")
    out_d = nc.dram_tensor("out", [C, N], F32, kind="ExternalOutput")

    persist = ctx.enter_context(tc.tile_pool(name="persist", bufs=1))
    stream = ctx.enter_context(tc.tile_pool(name="stream", bufs=2))

    xf = []
    for cc in range(4):
        t = persist.tile([P, N], F32, name=f"xf{cc}")
        nc.sync.dma_start(out=t, in_=x_f[cc * P : (cc + 1) * P, :])
        xf.append(t)
    wp = []
    for k in range(4):
        t = persist.tile([P, 384], F32R, name=f"wp{k}")
        nc.sync.dma_start(out=t, in_=wproj[k * P : (k + 1) * P, :])
        wp.append(t)
    wot = []
    for k in range(2):
        t = persist.tile([P, C], F32R, name=f"wot{k}")
        nc.sync.dma_start(out=t, in_=wo[k * P : (k + 1) * P, :])
        wot.append(t)
    ones_f = persist.tile([P, P], F32)
    nc.vector.memset(ones_f, 1.0)
    ones = persist.tile([P, P], F32R)
    nc.vector.tensor_copy(ones, ones_f)
    ident = persist.tile([P, P], F32)
    make_identity(nc, ident)

    theta = persist.tile([C8, N], F32R)
    phi = persist.tile([P, M], F32R)  # [64:128] pooled, [0:64] copy for K rows 0-63
    g = [persist.tile([P, M], F32, name=f"g{i}") for i in range(2)]
    gT = [persist.tile([P, C2], F32R, name=f"gT{mt}") for mt in range(MT)]

    # ---- phase 1: projections + pooling --------------------------------
    with tc.tile_pool(name="ppsum", bufs=2, space="PSUM") as pp:
        for ns in range(NS):
            nsl = slice(ns * SL, (ns + 1) * SL)
            msl = slice(ns * P, (ns + 1) * P)
            xr = []
            for k in range(4):
                t = stream.tile([P, SL], F32R, name="xr", tag=f"xr{k}")
                nc.sync.dma_start(out=t, in_=x_r[k * P : (k + 1) * P, nsl])
                xr.append(t)
            ps = [pp.tile([P, SL], F32, name="pp", tag=f"pp{mt}") for mt in range(3)]
            for mt in range(3):
                for k in range(4):
                    nc.tensor.matmul(
                        ps[mt],
                        lhsT=wp[k][:, mt * P : (mt + 1) * P],
                        rhs=xr[k],
                        start=(k == 0),
                        stop=(k == 3),
                    )
            nc.scalar.copy(out=theta[:, nsl], in_=ps[0][0:C8, :])
            nc.vector.tensor_reduce(
                out=phi[C8:P, msl],
                in_=_pool_view(ps[0][C8:P, :]),
                axis=AX.XY,
                op=ALU.max,
            )
            for i in range(2):
                nc.vector.tensor_reduce(
                    out=g[i][:, msl],
                    in_=_pool_view(ps[1 + i]),
                    axis=AX.XY,
                    op=ALU.max,
                )
            nc.sync.dma_start(out=phi[0:C8, msl], in_=phi[C8:P, msl])

    # ---- phase 1.5: transpose g -> gT [m, c2] --------------------------
    with tc.tile_pool(name="tpsum", bufs=2, space="PSUM") as tp:
        for i in range(2):
            for mt in range(MT):
                t = tp.tile([P, P], F32, name="tp", tag="tp")
                nc.tensor.transpose(t, g[i][:, mt * P : (mt + 1) * P], ident)
                nc.vector.tensor_copy(out=gT[mt][:, i * P : (i + 1) * P], in_=t)

    # ---- phase 2: scores / softmax / attend / project ------------------
    etp = ctx.enter_context(tc.tile_pool(name="et", bufs=2))
    miscp = ctx.enter_context(tc.tile_pool(name="misc", bufs=2))
    with tc.tile_pool(name="qpsum", bufs=2, space="PSUM") as qp:
        ET = [[None] * MT for _ in range(NS)]

        def emit_scores(i):
            nsl = slice(i * SL, (i + 1) * SL)
            for mt in range(MT):
                sp = qp.tile([P, SL], F32, name="sp", tag="s")
                nc.tensor.matmul(
                    sp,
                    lhsT=phi[0:C8, mt * P : (mt + 1) * P],
                    rhs=theta[:, nsl],
                    start=True,
                    stop=True,
                )
                et = etp.tile([P, SL], F32R, name="et", tag=f"et{mt}")
                nc.scalar.activation(et, sp, ACTF.Exp)
                ET[i][mt] = et

        emit_scores(0)
        for i in range(NS):
            if i + 1 < NS:
                emit_scores(i + 1)
            nsl = slice(i * SL, (i + 1) * SL)
            zp = qp.tile([P, SL], F32, name="zp", tag="z")
            ap = [qp.tile([P, SL], F32, name="ap", tag="a") for _ in range(2)]
            for mt in range(MT):
                st, sp_ = (mt == 0), (mt == MT - 1)
                nc.tensor.matmul(
                    zp, lhsT=ones, rhs=ET[i][mt], start=st, stop=sp_,
                    skip_group_check=True,
                )
                for ct in range(2):
                    nc.tensor.matmul(
                        ap[ct],
                        lhsT=gT[mt][:, ct * P : (ct + 1) * P],
                        rhs=ET[i][mt],
                        start=st,
                        stop=sp_,
                        skip_group_check=True,
                    )
            rinv = miscp.tile([P, SL], F32, name="rinv", tag="rinv")
            nc.vector.reciprocal(rinv, zp)
            att = []
            for ct in range(2):
                t = miscp.tile([P, SL], F32R, name="att", tag=f"att{ct}")
                nc.vector.tensor_mul(t, ap[ct], rinv)
                att.append(t)
            for ot in range(4):
                op_ = qp.tile([P, SL], F32, name="op", tag="o")
                for ct in range(2):
                    nc.tensor.matmul(
                        op_,
                        lhsT=wot[ct][:, ot * P : (ot + 1) * P],
                        rhs=att[ct],
                        start=(ct == 0),
                        stop=(ct == 1),
                    )
                ob = miscp.tile([P, SL], F32, name="ob", tag=f"ob{ot % 2}")
                nc.vector.tensor_add(ob, op_, xf[ot][:, nsl])
                nc.sync.dma_start(out=out_d[ot * P : (ot + 1) * P, nsl], in_=ob)


def build_nc():
    nc = bass.Bass(target_bir_lowering=False, trn_type="TRN2")
    with tile.TileContext(nc) as tc:
        with ExitStack() as ctx:
            emit(nc, tc, ctx)
    bass_rust.generate_event_semaphores(nc)
    return nc


def kernel(x, w_theta, w_phi, w_g, w_o, gamma):
    x = np.asarray(x, dtype=np.float32)
    B = x.shape[0]
    wproj = np.ascontiguousarray(
        np.concatenate(
            [np.asarray(w_theta).T, np.asarray(w_phi).T, np.asarray(w_g).T], axis=1
        ),
        dtype=np.float32,
    )
    wo_t = np.ascontiguousarray(
        (np.float32(gamma) * np.asarray(w_o)).T, dtype=np.float32
    )

    nc = build_nc()
    in_maps = []
    for b in range(B):
        xb = np.ascontiguousarray(x[b].reshape(C, N))
        in_maps.append({"x": xb, "x2": xb, "wproj": wproj, "wo": wo_t})
    res = run_bass_kernel_spmd(nc, in_maps, core_ids=list(range(B)))
    out = np.stack(
        [res.results[b]["out"].reshape(C, 64, 64) for b in range(B)]
    ).astype(np.float32)
    return out


# revision 15
# speedup vs baseline: 1.2805x; 1.2805x over previous
"""SAGAN-style self-attention block on 8 trn2 NeuronCores.

Full inputs: x [8, 512, 64, 64], w_theta [64, 512], w_phi [64, 512],
w_g [256, 512], w_o [512, 256], gamma scalar.

Sharding: data-parallel over batch — one batch item per core. Each core runs
an identical Bass program over its own x[b]; weights are replicated.

Per-core math (C=512, n=H*W=4096, m=n/4=1024):
  theta = w_theta @ x            [64, 4096]
  phi   = pool2(w_phi @ x)       [64, 1024]
  g     = pool2(w_g @ x)         [256, 1024]
  S^T   = phi^T @ theta          [1024, 4096]   (scores, transposed layout)
  E     = exp(S^T)               (no max-subtraction needed: |S| < ~50)
  Z     = ones^T @ E             [*, 4096]      (row sums, broadcast layout)
  att   = (g @ E) / Z            [256, 4096]
  out   = (gamma*w_o) @ att + x  [512, 4096]

All matmuls run as float32r (full-rate fp32 on the PE at N>=512; tf32-like
input rounding, ~1.5e-4 rel err). The residual add uses unrounded fp32 x.
"""

from contextlib import ExitStack

import numpy as np

import bass_rust
import concourse.bass as bass
import concourse.mybir as mybir
import concourse.tile as tile
from concourse.bass_utils import run_bass_kernel_spmd
from concourse.masks import make_identity

P = 128
C = 512  # channels
C8 = 64  # theta/phi channels
C2 = 256  # g channels
N = 4096  # H*W
M = 1024  # pooled spatial
NS = 8  # n-slices
SL = 512  # n-slice width
MT = 8  # m-tiles of 128
F32 = mybir.dt.float32
F32R = mybir.dt.float32r
AX = mybir.AxisListType
ALU = mybir.AluOpType
ACTF = mybir.ActivationFunctionType


def _pool_view(ap):
    """[p, 512] slice of the conv output -> 5D maxpool view [p, h2, w2, dy, dx].

    Within an n-slice of 512 = 8 image rows: local n = (2*h2+dy)*64 + 2*w2+dx.
    """
    return ap.rearrange("p (h2 dy w2 dx) -> p h2 w2 dy dx", h2=4, dy=2, w2=32, dx=2)


def emit(nc, tc, ctx):
    x_f = nc.dram_tensor("x", [C, N], F32, kind="ExternalInput")
    wproj = nc.dram_tensor("wproj", [C, 384], F32R, kind="ExternalInput")
    wo = nc.dram_tensor("wo", [C2, C], F32R, kind="ExternalInput")
    out_d = nc.dram_tensor("out", [C, N], F32, kind="ExternalOutput")

    persist = ctx.enter_context(tc.tile_pool(name="persist", bufs=1))
    stream = ctx.enter_context(tc.tile_pool(name="stream", bufs=2))

    wpt = persist.tile([P, 4, 384], F32R, name="wpt")
    nc.sync.dma_start(out=wpt, in_=wproj.ap().rearrange("(k p) o -> p k o", k=4))
    wp = [wpt[:, k, :] for k in range(4)]
    ones_f = persist.tile([P, P], F32)
    nc.vector.memset(ones_f, 1.0)
    ones = persist.tile([P, P], F32R)
    nc.vector.tensor_copy(ones, ones_f)
    ident = persist.tile([P, P], F32)
    make_identity(nc, ident)

    # x loads: slice-major chunks so phase-1 slice 0 unblocks after ~1MB
    xf = [persist.tile([P, N], F32, name=f"xf{cc}") for cc in range(4)]
    for q in range(NS):
        for cc in range(4):
            nc.sync.dma_start(
                out=xf[cc][:, q * SL : (q + 1) * SL],
                in_=x_f[cc * P : (cc + 1) * P, q * SL : (q + 1) * SL],
            )
    wot = []
    for k in range(2):
        t = persist.tile([P, C], F32R, name=f"wot{k}")
        nc.sync.dma_start(out=t, in_=wo[k * P : (k + 1) * P, :])
        wot.append(t)

    theta = persist.tile([C8, N], F32R)
    phi = persist.tile([P, M], F32R)  # [64:128] pooled, [0:64] copy for K rows 0-63
    g = [persist.tile([P, M], F32, name=f"g{i}") for i in range(2)]
    gT = [persist.tile([P, C2], F32R, name=f"gT{mt}") for mt in range(MT)]

    # ---- phase 1: projections + pooling + g transposes -----------------
    with tc.tile_pool(name="ppsum", bufs=2, space="PSUM") as pp, tc.tile_pool(
        name="tpsum", bufs=2, space="PSUM"
    ) as tp:
        for ns in range(NS):
            nsl = slice(ns * SL, (ns + 1) * SL)
            msl = slice(ns * P, (ns + 1) * P)
            xr = []
            for k in range(4):
                t = stream.tile([P, SL], F32R, name="xr", tag=f"xr{k}")
                if k == 0:
                    nc.vector.tensor_copy(t, xf[k][:, nsl])
                else:
                    nc.scalar.copy(out=t, in_=xf[k][:, nsl])
                xr.append(t)
            ps = [pp.tile([P, SL], F32, name="pp", tag=f"pp{mt}") for mt in range(3)]
            for mt in range(3):
                for k in range(4):
                    nc.tensor.matmul(
                        ps[mt],
                        lhsT=wp[k][:, mt * P : (mt + 1) * P],
                        rhs=xr[k],
                        start=(k == 0),
                        stop=(k == 3),
                    )
            nc.scalar.copy(out=theta[:, nsl], in_=ps[0][0:C8, :])
            nc.vector.tensor_reduce(
                out=phi[C8:P, msl],
                in_=_pool_view(ps[0][C8:P, :]),
                axis=AX.XY,
                op=ALU.max,
            )
            for i in range(2):
                nc.vector.tensor_reduce(
                    out=g[i][:, msl],
                    in_=_pool_view(ps[1 + i]),
                    axis=AX.XY,
                    op=ALU.max,
                )
            # transpose this slice's pooled g columns into gT[ns]
            for i in range(2):
                t = tp.tile([P, P], F32, name="tp", tag="tp")
                nc.tensor.transpose(t, g[i][:, msl], ident)
                nc.vector.tensor_copy(out=gT[ns][:, i * P : (i + 1) * P], in_=t)
            nc.sync.dma_start(out=phi[0:C8, msl], in_=phi[C8:P, msl])

    # ---- phase 2: scores / softmax / attend / project ------------------
    etp = ctx.enter_context(tc.tile_pool(name="et", bufs=2))
    miscp = ctx.enter_context(tc.tile_pool(name="misc", bufs=2))
    with tc.tile_pool(name="qpsum", bufs=2, space="PSUM") as qp:
        ET = [[None] * MT for _ in range(NS)]

        def emit_scores(i):
            nsl = slice(i * SL, (i + 1) * SL)
            for mt in range(MT):
                sp = qp.tile([P, SL], F32, name="sp", tag="s")
                nc.tensor.matmul(
                    sp,
                    lhsT=phi[0:C8, mt * P : (mt + 1) * P],
                    rhs=theta[:, nsl],
                    start=True,
                    stop=True,
                )
                et = etp.tile([P, SL], F32R, name="et", tag=f"et{mt}")
                nc.scalar.activation(et, sp, ACTF.Exp)
                ET[i][mt] = et

        emit_scores(0)
        for i in range(NS):
            if i + 1 < NS:
                emit_scores(i + 1)
            nsl = slice(i * SL, (i + 1) * SL)
            zp = qp.tile([P, SL], F32, name="zp", tag="z")
            ap = [qp.tile([P, SL], F32, name="ap", tag="a") for _ in range(2)]
            for mt in range(MT):
                st, sp_ = (mt == 0), (mt == MT - 1)
                nc.tensor.matmul(
                    zp, lhsT=ones, rhs=ET[i][mt], start=st, stop=sp_,
                    skip_group_check=True,
                )
                for ct in range(2):
                    nc.tensor.matmul(
                        ap[ct],
                        lhsT=gT[mt][:, ct * P : (ct + 1) * P],
                        rhs=ET[i][mt],
                        start=st,
                        stop=sp_,
                        skip_group_check=True,
                    )
            rinv = miscp.tile([P, SL], F32, name="rinv", tag="rinv")
            nc.vector.reciprocal(rinv, zp)
            att = []
            for ct in range(2):
                t = miscp.tile([P, SL], F32R, name="att", tag=f"att{ct}")
                nc.vector.tensor_mul(t, ap[ct], rinv)
                att.append(t)
            ob = miscp.tile([P, 4, SL], F32, name="ob", tag="ob")
            for ot in range(4):
                op_ = qp.tile([P, SL], F32, name="op", tag="o")
                for ct in range(2):
                    nc.tensor.matmul(
                        op_,
                        lhsT=wot[ct][:, ot * P : (ot + 1) * P],
                        rhs=att[ct],
                        start=(ct == 0),
                        stop=(ct == 1),
                    )
                nc.vector.tensor_add(ob[:, ot, :], op_, xf[ot][:, nsl])
            # single DMA for the whole [512, SL] output slice
            nc.sync.dma_start(
                out=out_d[:, nsl].rearrange("(ct p) n -> p ct n", ct=4),
                in_=ob,
            )


def build_nc():
    nc = bass.Bass(target_bir_lowering=False, trn_type="TRN2")
    with tile.TileContext(nc) as tc:
        with ExitStack() as ctx:
            emit(nc, tc, ctx)
    bass_rust.generate_event_semaphores(nc)
    return nc


def kernel(x, w_theta, w_phi, w_g, w_o, gamma):
    x = np.asarray(x, dtype=np.float32)
    B = x.shape[0]
    wproj = np.ascontiguousarray(
        np.concatenate(
            [np.asarray(w_theta).T, np.asarray(w_phi).T, np.asarray(w_g).T], axis=1
        ),
        dtype=np.float32,
    )
    wo_t = np.ascontiguousarray(
        (np.float32(gamma) * np.asarray(w_o)).T, dtype=np.float32
    )

    nc = build_nc()
    in_maps = []
    for b in range(B):
        xb = np.ascontiguousarray(x[b].reshape(C, N))
        in_maps.append({"x": xb, "wproj": wproj, "wo": wo_t})
    res = run_bass_kernel_spmd(nc, in_maps, core_ids=list(range(B)))
    out = np.stack(
        [res.results[b]["out"].reshape(C, 64, 64) for b in range(B)]
    ).astype(np.float32)
    return out


# revision 16
# speedup vs baseline: 1.2926x; 1.0095x over previous
"""SAGAN-style self-attention block on 8 trn2 NeuronCores.

Full inputs: x [8, 512, 64, 64], w_theta [64, 512], w_phi [64, 512],
w_g [256, 512], w_o [512, 256], gamma scalar.

Sharding: data-parallel over batch — one batch item per core. Each core runs
an identical Bass program over its own x[b]; weights are replicated.

Per-core math (C=512, n=H*W=4096, m=n/4=1024):
  theta = w_theta @ x            [64, 4096]
  phi   = pool2(w_phi @ x)       [64, 1024]
  g     = pool2(w_g @ x)         [256, 1024]
  S^T   = phi^T @ theta          [1024, 4096]   (scores, transposed layout)
  E     = exp(S^T)               (no max-subtraction needed: |S| < ~50)
  Z     = ones^T @ E             [*, 4096]      (row sums, broadcast layout)
  att   = (g @ E) / Z            [256, 4096]
  out   = (gamma*w_o) @ att + x  [512, 4096]

All matmuls run as float32r (full-rate fp32 on the PE at N>=512; tf32-like
input rounding, ~1.5e-4 rel err). The residual add uses unrounded fp32 x.
"""

from contextlib import ExitStack

import numpy as np

import bass_rust
import concourse.bass as bass
import concourse.mybir as mybir
import concourse.tile as tile
from concourse.bass_utils import run_bass_kernel_spmd
from concourse.masks import make_identity

P = 128
C = 512  # channels
C8 = 64  # theta/phi channels
C2 = 256  # g channels
N = 4096  # H*W
M = 1024  # pooled spatial
NS = 8  # n-slices
SL = 512  # n-slice width
MT = 8  # m-tiles of 128
F32 = mybir.dt.float32
F32R = mybir.dt.float32r
AX = mybir.AxisListType
ALU = mybir.AluOpType
ACTF = mybir.ActivationFunctionType


def _pool_view(ap):
    """[p, 512] slice of the conv output -> 5D maxpool view [p, h2, w2, dy, dx].

    Within an n-slice of 512 = 8 image rows: local n = (2*h2+dy)*64 + 2*w2+dx.
    """
    return ap.rearrange("p (h2 dy w2 dx) -> p h2 w2 dy dx", h2=4, dy=2, w2=32, dx=2)


def emit(nc, tc, ctx):
    x_f = nc.dram_tensor("x", [C, N], F32, kind="ExternalInput")
    wproj = nc.dram_tensor("wproj", [C, 384], F32R, kind="ExternalInput")
    wo = nc.dram_tensor("wo", [C2, C], F32R, kind="ExternalInput")
    out_d = nc.dram_tensor("out", [C, N], F32, kind="ExternalOutput")

    persist = ctx.enter_context(tc.tile_pool(name="persist", bufs=1))
    stream = ctx.enter_context(tc.tile_pool(name="stream", bufs=2))

    wpt = persist.tile([P, 4, 384], F32R, name="wpt")
    nc.sync.dma_start(out=wpt, in_=wproj.ap().rearrange("(k p) o -> p k o", k=4))
    wp = [wpt[:, k, :] for k in range(4)]
    ones_f = persist.tile([P, P], F32)
    nc.vector.memset(ones_f, 1.0)
    ones = persist.tile([P, P], F32R)
    nc.vector.tensor_copy(ones, ones_f)
    ident = persist.tile([P, P], F32)
    make_identity(nc, ident)

    # x loads: slice-major chunks so phase-1 slice 0 unblocks after ~1MB
    xf = [persist.tile([P, N], F32, name=f"xf{cc}") for cc in range(4)]
    for q in range(NS):
        for cc in range(4):
            nc.sync.dma_start(
                out=xf[cc][:, q * SL : (q + 1) * SL],
                in_=x_f[cc * P : (cc + 1) * P, q * SL : (q + 1) * SL],
            )
    wot = []
    for k in range(2):
        t = persist.tile([P, C], F32R, name=f"wot{k}")
        nc.sync.dma_start(out=t, in_=wo[k * P : (k + 1) * P, :])
        wot.append(t)

    theta = persist.tile([C8, N], F32R)
    phi = persist.tile([P, M], F32R)  # [64:128] pooled, [0:64] copy for K rows 0-63
    g = [persist.tile([P, M], F32, name=f"g{i}") for i in range(2)]
    gT = [persist.tile([P, C2], F32R, name=f"gT{mt}") for mt in range(MT)]

    # ---- phase 1: projections + pooling + g transposes -----------------
    with tc.tile_pool(name="ppsum", bufs=2, space="PSUM") as pp, tc.tile_pool(
        name="tpsum", bufs=2, space="PSUM"
    ) as tp:
        for ns in range(NS):
            nsl = slice(ns * SL, (ns + 1) * SL)
            msl = slice(ns * P, (ns + 1) * P)
            xr = []
            for k in range(4):
                t = stream.tile([P, SL], F32R, name="xr", tag=f"xr{k}")
                if k == 0:
                    nc.vector.tensor_copy(t, xf[k][:, nsl])
                else:
                    nc.scalar.copy(out=t, in_=xf[k][:, nsl])
                xr.append(t)
            ps = [pp.tile([P, SL], F32, name="pp", tag=f"pp{mt}") for mt in range(3)]
            for mt in range(3):
                for k in range(4):
                    nc.tensor.matmul(
                        ps[mt],
                        lhsT=wp[k][:, mt * P : (mt + 1) * P],
                        rhs=xr[k],
                        start=(k == 0),
                        stop=(k == 3),
                    )
            nc.scalar.copy(out=theta[:, nsl], in_=ps[0][0:C8, :])
            nc.vector.tensor_reduce(
                out=phi[C8:P, msl],
                in_=_pool_view(ps[0][C8:P, :]),
                axis=AX.XY,
                op=ALU.max,
            )
            for i in range(2):
                nc.vector.tensor_reduce(
                    out=g[i][:, msl],
                    in_=_pool_view(ps[1 + i]),
                    axis=AX.XY,
                    op=ALU.max,
                )
            # transpose this slice's pooled g columns into gT[ns]
            for i in range(2):
                t = tp.tile([P, P], F32, name="tp", tag="tp")
                nc.tensor.transpose(t, g[i][:, msl], ident)
                nc.vector.tensor_copy(out=gT[ns][:, i * P : (i + 1) * P], in_=t)
            nc.sync.dma_start(out=phi[0:C8, msl], in_=phi[C8:P, msl])

    # ---- phase 2: scores / softmax / attend / project ------------------
    etp = ctx.enter_context(tc.tile_pool(name="et", bufs=2))
    miscp = ctx.enter_context(tc.tile_pool(name="misc", bufs=2))
    with tc.tile_pool(name="qpsum", bufs=2, space="PSUM") as qp:
        ET = [[None] * MT for _ in range(NS)]

        def emit_scores(i):
            nsl = slice(i * SL, (i + 1) * SL)
            # duplicate theta slice to partitions 64-127 so odd m-tiles can run
            # concurrently in PE rows 64-127 (row tiling)
            th2 = miscp.tile([P, SL], F32R, name="th2", tag="th2")
            nc.sync.dma_start(out=th2[C8:P, :], in_=theta[:, nsl])
            for mt in range(MT):
                sp = qp.tile([P, SL], F32, name="sp", tag="s")
                if mt % 2 == 0:
                    nc.tensor.matmul(
                        sp,
                        lhsT=phi[0:C8, mt * P : (mt + 1) * P],
                        rhs=theta[:, nsl],
                        start=True,
                        stop=True,
                    )
                else:
                    nc.tensor.matmul(
                        sp,
                        lhsT=phi[C8:P, mt * P : (mt + 1) * P],
                        rhs=th2[C8:P, :],
                        start=True,
                        stop=True,
                        tile_position=(C8, 0),
                    )
                et = etp.tile([P, SL], F32R, name="et", tag=f"et{mt}")
                nc.scalar.activation(et, sp, ACTF.Exp)
                ET[i][mt] = et

        emit_scores(0)
        for i in range(NS):
            if i + 1 < NS:
                emit_scores(i + 1)
            nsl = slice(i * SL, (i + 1) * SL)
            zp = qp.tile([P, SL], F32, name="zp", tag="z")
            ap = [qp.tile([P, SL], F32, name="ap", tag="a") for _ in range(2)]
            for mt in range(MT):
                st, sp_ = (mt == 0), (mt == MT - 1)
                nc.tensor.matmul(
                    zp, lhsT=ones, rhs=ET[i][mt], start=st, stop=sp_,
                    skip_group_check=True,
                )
                for ct in range(2):
                    nc.tensor.matmul(
                        ap[ct],
                        lhsT=gT[mt][:, ct * P : (ct + 1) * P],
                        rhs=ET[i][mt],
                        start=st,
                        stop=sp_,
                        skip_group_check=True,
                    )
            rinv = miscp.tile([P, SL], F32, name="rinv", tag="rinv")
            nc.vector.reciprocal(rinv, zp)
            att = []
            for ct in range(2):
                t = miscp.tile([P, SL], F32R, name="att", tag=f"att{ct}")
                nc.vector.tensor_mul(t, ap[ct], rinv)
                att.append(t)
            for ot in range(4):
                op_ = qp.tile([P, SL], F32, name="op", tag="o")
                for ct in range(2):
                    nc.tensor.matmul(
                        op_,
                        lhsT=wot[ct][:, ot * P : (ot + 1) * P],
                        rhs=att[ct],
                        start=(ct == 0),
                        stop=(ct == 1),
                    )
                ob = miscp.tile([P, SL], F32, name="ob", tag=f"ob{ot % 2}")
                nc.vector.tensor_add(ob, op_, xf[ot][:, nsl])
                nc.sync.dma_start(out=out_d[ot * P : (ot + 1) * P, nsl], in_=ob)


def build_nc():
    nc = bass.Bass(target_bir_lowering=False, trn_type="TRN2")
    with tile.TileContext(nc) as tc:
        with ExitStack() as ctx:
            emit(nc, tc, ctx)
    bass_rust.generate_event_semaphores(nc)
    return nc


def kernel(x, w_theta, w_phi, w_g, w_o, gamma):
    x = np.asarray(x, dtype=np.float32)
    B = x.shape[0]
    wproj = np.ascontiguousarray(
        np.concatenate(
            [np.asarray(w_theta).T, np.asarray(w_phi).T, np.asarray(w_g).T], axis=1
        ),
        dtype=np.float32,
    )
    wo_t = np.ascontiguousarray(
        (np.float32(gamma) * np.asarray(w_o)).T, dtype=np.float32
    )

    nc = build_nc()
    in_maps = []
    for b in range(B):
        xb = np.ascontiguousarray(x[b].reshape(C, N))
        in_maps.append({"x": xb, "wproj": wproj, "wo": wo_t})
    res = run_bass_kernel_spmd(nc, in_maps, core_ids=list(range(B)))
    out = np.stack(
        [res.results[b]["out"].reshape(C, 64, 64) for b in range(B)]
    ).astype(np.float32)
    return out


# revision 17
# speedup vs baseline: 1.3135x; 1.0162x over previous
"""SAGAN-style self-attention block on 8 trn2 NeuronCores.

Full inputs: x [8, 512, 64, 64], w_theta [64, 512], w_phi [64, 512],
w_g [256, 512], w_o [512, 256], gamma scalar.

Sharding: data-parallel over batch — one batch item per core. Each core runs
an identical Bass program over its own x[b]; weights are replicated.

Per-core math (C=512, n=H*W=4096, m=n/4=1024):
  theta = w_theta @ x            [64, 4096]
  phi   = pool2(w_phi @ x)       [64, 1024]
  g     = pool2(w_g @ x)         [256, 1024]
  S^T   = phi^T @ theta          [1024, 4096]   (scores, transposed layout)
  E     = exp(S^T)               (no max-subtraction needed: |S| < ~50)
  Z     = ones^T @ E             [*, 4096]      (row sums, broadcast layout)
  att   = (g @ E) / Z            [256, 4096]
  out   = (gamma*w_o) @ att + x  [512, 4096]

All matmuls run as float32r (full-rate fp32 on the PE at N>=512; tf32-like
input rounding, ~1.5e-4 rel err). The residual add uses unrounded fp32 x.
"""

from contextlib import ExitStack

import numpy as np

import bass_rust
import concourse.bass as bass
import concourse.mybir as mybir
import concourse.tile as tile
from concourse.bass_utils import run_bass_kernel_spmd
from concourse.masks import make_identity

P = 128
C = 512  # channels
C8 = 64  # theta/phi channels
C2 = 256  # g channels
N = 4096  # H*W
M = 1024  # pooled spatial
NS = 8  # n-slices
SL = 512  # n-slice width
MT = 8  # m-tiles of 128
F32 = mybir.dt.float32
F32R = mybir.dt.float32r
AX = mybir.AxisListType
ALU = mybir.AluOpType
ACTF = mybir.ActivationFunctionType


def _pool_view(ap):
    """[p, 512] slice of the conv output -> 5D maxpool view [p, h2, w2, dy, dx].

    Within an n-slice of 512 = 8 image rows: local n = (2*h2+dy)*64 + 2*w2+dx.
    """
    return ap.rearrange("p (h2 dy w2 dx) -> p h2 w2 dy dx", h2=4, dy=2, w2=32, dx=2)


def emit(nc, tc, ctx):
    x_f = nc.dram_tensor("x", [C, N], F32, kind="ExternalInput")
    wproj = nc.dram_tensor("wproj", [C, 384], F32R, kind="ExternalInput")
    wo = nc.dram_tensor("wo", [C2, C], F32R, kind="ExternalInput")
    out_d = nc.dram_tensor("out", [C, N], F32, kind="ExternalOutput")

    persist = ctx.enter_context(tc.tile_pool(name="persist", bufs=1))
    stream = ctx.enter_context(tc.tile_pool(name="stream", bufs=2))

    wpt = persist.tile([P, 4, 384], F32R, name="wpt")
    nc.sync.dma_start(out=wpt, in_=wproj.ap().rearrange("(k p) o -> p k o", k=4))
    wp = [wpt[:, k, :] for k in range(4)]
    ones_f = persist.tile([P, P], F32)
    nc.vector.memset(ones_f, 1.0)
    ones = persist.tile([P, P], F32R)
    nc.vector.tensor_copy(ones, ones_f)
    ident = persist.tile([P, P], F32)
    make_identity(nc, ident)

    # x loads: slice-major chunks so phase-1 slice 0 unblocks after ~1MB
    xf = [persist.tile([P, N], F32, name=f"xf{cc}") for cc in range(4)]
    for q in range(NS):
        for cc in range(4):
            nc.sync.dma_start(
                out=xf[cc][:, q * SL : (q + 1) * SL],
                in_=x_f[cc * P : (cc + 1) * P, q * SL : (q + 1) * SL],
            )
    wot = []
    for k in range(2):
        t = persist.tile([P, C], F32R, name=f"wot{k}")
        nc.sync.dma_start(out=t, in_=wo[k * P : (k + 1) * P, :])
        wot.append(t)

    theta = persist.tile([C8, N], F32R)
    phi = persist.tile([P, M], F32R)  # [64:128] pooled, [0:64] copy for K rows 0-63
    g = [persist.tile([P, M], F32, name=f"g{i}") for i in range(2)]
    gT = [persist.tile([P, C2], F32R, name=f"gT{mt}") for mt in range(MT)]

    # score psum pool lives across phases 1+2 so slice-0 scores can start
    # inside phase 1
    spool = ctx.enter_context(tc.tile_pool(name="spsum", bufs=2, space="PSUM"))
    etp = ctx.enter_context(tc.tile_pool(name="et", bufs=3))
    miscp = ctx.enter_context(tc.tile_pool(name="misc", bufs=2))
    ET = [[None] * MT for _ in range(NS)]
    TH2 = [None] * NS

    def emit_th2(i):
        nsl = slice(i * SL, (i + 1) * SL)
        t = miscp.tile([P, SL], F32R, name="th2", tag="th2", bufs=3)
        nc.sync.dma_start(out=t[C8:P, :], in_=theta[:, nsl])
        TH2[i] = t

    def emit_score(i, mt):
        nsl = slice(i * SL, (i + 1) * SL)
        sp = spool.tile([P, SL], F32, name="sp", tag="s")
        if mt % 2 == 0:
            nc.tensor.matmul(
                sp,
                lhsT=phi[0:C8, mt * P : (mt + 1) * P],
                rhs=theta[:, nsl],
                start=True,
                stop=True,
            )
        else:
            nc.tensor.matmul(
                sp,
                lhsT=phi[C8:P, mt * P : (mt + 1) * P],
                rhs=TH2[i][C8:P, :],
                start=True,
                stop=True,
                tile_position=(C8, 0),
            )
        et = etp.tile([P, SL], F32R, name="et", tag=f"et{mt}")
        nc.scalar.activation(et, sp, ACTF.Exp)
        ET[i][mt] = et

    # ---- phase 1: projections + pooling + g transposes -----------------
    with tc.tile_pool(name="ppsum", bufs=2, space="PSUM") as pp, tc.tile_pool(
        name="tpsum", bufs=1, space="PSUM"
    ) as tp:
        for ns in range(NS):
            nsl = slice(ns * SL, (ns + 1) * SL)
            msl = slice(ns * P, (ns + 1) * P)
            xr = []
            for k in range(4):
                t = stream.tile([P, SL], F32R, name="xr", tag=f"xr{k}")
                if k == 0:
                    nc.vector.tensor_copy(t, xf[k][:, nsl])
                else:
                    nc.scalar.copy(out=t, in_=xf[k][:, nsl])
                xr.append(t)
            ps = [
                pp.tile(
                    [P, SL], F32, name="pp", tag=f"pp{mt}",
                    bufs=(1 if mt == 2 else 2),
                )
                for mt in range(3)
            ]
            for mt in range(3):
                for k in range(4):
                    nc.tensor.matmul(
                        ps[mt],
                        lhsT=wp[k][:, mt * P : (mt + 1) * P],
                        rhs=xr[k],
                        start=(k == 0),
                        stop=(k == 3),
                    )
            nc.scalar.copy(out=theta[:, nsl], in_=ps[0][0:C8, :])
            nc.vector.tensor_reduce(
                out=phi[C8:P, msl],
                in_=_pool_view(ps[0][C8:P, :]),
                axis=AX.XY,
                op=ALU.max,
            )
            for i in range(2):
                nc.vector.tensor_reduce(
                    out=g[i][:, msl],
                    in_=_pool_view(ps[1 + i]),
                    axis=AX.XY,
                    op=ALU.max,
                )
            # transpose this slice's pooled g columns into gT[ns]
            for i in range(2):
                t = tp.tile([P, P], F32, name="tp", tag="tp")
                nc.tensor.transpose(t, g[i][:, msl], ident)
                nc.vector.tensor_copy(out=gT[ns][:, i * P : (i + 1) * P], in_=t)
            nc.sync.dma_start(out=phi[0:C8, msl], in_=phi[C8:P, msl])
            if ns == 0:
                emit_th2(0)
            emit_score(0, ns)

    # ---- phase 2: softmax / attend / project ---------------------------
    with tc.tile_pool(name="qpsum", bufs=2, space="PSUM") as qp:
        def emit_scores(i):
            emit_th2(i)
            for mt in range(MT):
                emit_score(i, mt)

        emit_scores(1)
        for i in range(NS):
            if i + 2 < NS:
                emit_scores(i + 2)
            nsl = slice(i * SL, (i + 1) * SL)
            zp = qp.tile([P, SL], F32, name="zp", tag="z")
            ap = [qp.tile([P, SL], F32, name="ap", tag="a") for _ in range(2)]
            for mt in range(MT):
                st, sp_ = (mt == 0), (mt == MT - 1)
                nc.tensor.matmul(
                    zp, lhsT=ones, rhs=ET[i][mt], start=st, stop=sp_,
                    skip_group_check=True,
                )
                for ct in range(2):
                    nc.tensor.matmul(
                        ap[ct],
                        lhsT=gT[mt][:, ct * P : (ct + 1) * P],
                        rhs=ET[i][mt],
                        start=st,
                        stop=sp_,
                        skip_group_check=True,
                    )
            rinv = miscp.tile([P, SL], F32, name="rinv", tag="rinv")
            nc.vector.reciprocal(rinv, zp)
            att = []
            for ct in range(2):
                t = miscp.tile([P, SL], F32R, name="att", tag=f"att{ct}")
                nc.vector.tensor_mul(t, ap[ct], rinv)
                att.append(t)
            for ot in range(4):
                op_ = qp.tile([P, SL], F32, name="op", tag="o")
                for ct in range(2):
                    nc.tensor.matmul(
                        op_,
                        lhsT=wot[ct][:, ot * P : (ot + 1) * P],
                        rhs=att[ct],
                        start=(ct == 0),
                        stop=(ct == 1),
                    )
                ob = miscp.tile([P, SL], F32, name="ob", tag=f"ob{ot % 2}")
                nc.vector.tensor_add(ob, op_, xf[ot][:, nsl])
                nc.sync.dma_start(out=out_d[ot * P : (ot + 1) * P, nsl], in_=ob)


def build_nc():
    nc = bass.Bass(target_bir_lowering=False, trn_type="TRN2")
    with tile.TileContext(nc) as tc:
        with ExitStack() as ctx:
            emit(nc, tc, ctx)
    bass_rust.generate_event_semaphores(nc)
    return nc


def kernel(x, w_theta, w_phi, w_g, w_o, gamma):
    x = np.asarray(x, dtype=np.float32)
    B = x.shape[0]
    wproj = np.ascontiguousarray(
        np.concatenate(
            [np.asarray(w_theta).T, np.asarray(w_phi).T, np.asarray(w_g).T], axis=1
        ),
        dtype=np.float32,
    )
    wo_t = np.ascontiguousarray(
        (np.float32(gamma) * np.asarray(w_o)).T, dtype=np.float32
    )

    nc = build_nc()
    in_maps = []
    for b in range(B):
        xb = np.ascontiguousarray(x[b].reshape(C, N))
        in_maps.append({"x": xb, "wproj": wproj, "wo": wo_t})
    res = run_bass_kernel_spmd(nc, in_maps, core_ids=list(range(B)))
    out = np.stack(
        [res.results[b]["out"].reshape(C, 64, 64) for b in range(B)]
    ).astype(np.float32)
    return out


# revision 25
# speedup vs baseline: 1.3628x; 1.0376x over previous
"""SAGAN-style self-attention block on 8 trn2 NeuronCores.

Full inputs: x [8, 512, 64, 64], w_theta [64, 512], w_phi [64, 512],
w_g [256, 512], w_o [512, 256], gamma scalar.

Sharding: data-parallel over batch — one batch item per core. Each core runs
an identical Bass program over its own x[b]; weights are replicated.

Per-core math (C=512, n=H*W=4096, m=n/4=1024):
  theta = w_theta @ x            [64, 4096]
  phi   = pool2(w_phi @ x)       [64, 1024]
  g     = pool2(w_g @ x)         [256, 1024]
  S^T   = phi^T @ theta          [1024, 4096]   (scores, transposed layout)
  E     = exp(S^T)               (no max-subtraction needed: |S| < ~50)
  Z     = ones^T @ E             [*, 4096]      (row sums, broadcast layout)
  att   = (g @ E) / Z            [256, 4096]
  out   = (gamma*w_o) @ att + x  [512, 4096]

All matmuls run as float32r (full-rate fp32 on the PE at N>=512; tf32-like
input rounding, ~1.5e-4 rel err). The residual add uses unrounded fp32 x.
"""

from contextlib import ExitStack

import numpy as np

import bass_rust
import concourse.bass as bass
import concourse.mybir as mybir
import concourse.tile as tile
from concourse.bass_utils import run_bass_kernel_spmd
from concourse.masks import make_identity

P = 128
C = 512  # channels
C8 = 64  # theta/phi channels
C2 = 256  # g channels
N = 4096  # H*W
M = 1024  # pooled spatial
NS = 8  # n-slices
SL = 512  # n-slice width
MT = 8  # m-tiles of 128
F32 = mybir.dt.float32
F32R = mybir.dt.float32r
AX = mybir.AxisListType
ALU = mybir.AluOpType
ACTF = mybir.ActivationFunctionType


def _pool_view(ap):
    """[p, 512] slice of the conv output -> 5D maxpool view [p, h2, w2, dy, dx].

    Within an n-slice of 512 = 8 image rows: local n = (2*h2+dy)*64 + 2*w2+dx.
    """
    return ap.rearrange("p (h2 dy w2 dx) -> p h2 w2 dy dx", h2=4, dy=2, w2=32, dx=2)


def emit(nc, tc, ctx):
    x_f = nc.dram_tensor("x", [C, N], F32, kind="ExternalInput")
    wproj = nc.dram_tensor("wproj", [C, 384], F32R, kind="ExternalInput")
    wo = nc.dram_tensor("wo", [C2, C], F32R, kind="ExternalInput")
    out_d = nc.dram_tensor("out", [C, N], F32, kind="ExternalOutput")

    persist = ctx.enter_context(tc.tile_pool(name="persist", bufs=1))
    stream = ctx.enter_context(tc.tile_pool(name="stream", bufs=2))

    wpt = persist.tile([P, 4, 384], F32R, name="wpt")
    nc.sync.dma_start(out=wpt, in_=wproj.ap().rearrange("(k p) o -> p k o", k=4))
    wp = [wpt[:, k, :] for k in range(4)]
    ones_f = persist.tile([P, P], F32)
    nc.vector.memset(ones_f, 1.0)
    ones = persist.tile([P, P], F32R)
    nc.vector.tensor_copy(ones, ones_f)
    ident = persist.tile([P, P], F32)
    make_identity(nc, ident)

    # x loads: slice-major chunks so phase-1 slice 0 unblocks after ~1MB
    xf = [persist.tile([P, N], F32, name=f"xf{cc}") for cc in range(4)]
    for q in range(NS):
        for cc in range(4):
            nc.sync.dma_start(
                out=xf[cc][:, q * SL : (q + 1) * SL],
                in_=x_f[cc * P : (cc + 1) * P, q * SL : (q + 1) * SL],
            )
    wot = []
    for k in range(2):
        t = persist.tile([P, C], F32R, name=f"wot{k}")
        nc.sync.dma_start(out=t, in_=wo[k * P : (k + 1) * P, :])
        wot.append(t)

    theta = persist.tile([C8, N], F32R)
    phi = persist.tile([P, M], F32R)  # [64:128] pooled, [0:64] copy for K rows 0-63
    g = [persist.tile([P, M], F32, name=f"g{i}") for i in range(2)]
    gT = [persist.tile([P, C2], F32R, name=f"gT{mt}") for mt in range(MT)]

    # score psum pool lives across phases 1+2 so slice-0 scores can start
    # inside phase 1
    spool = ctx.enter_context(tc.tile_pool(name="spsum", bufs=2, space="PSUM"))
    etp = ctx.enter_context(tc.tile_pool(name="et", bufs=3))
    miscp = ctx.enter_context(tc.tile_pool(name="misc", bufs=2))
    ET = [[None] * MT for _ in range(NS)]
    TH2 = [None] * NS

    def emit_th2(i):
        nsl = slice(i * SL, (i + 1) * SL)
        t = miscp.tile([P, SL], F32R, name="th2", tag="th2", bufs=3)
        nc.sync.dma_start(out=t[C8:P, :], in_=theta[:, nsl])
        TH2[i] = t

    def emit_score_pair(i, j):
        # m-tiles 2j and 2j+1 run concurrently via row tiling (separate banks)
        nsl = slice(i * SL, (i + 1) * SL)
        for half, mt in enumerate((2 * j, 2 * j + 1)):
            sp = spool.tile([P, SL], F32, name="sp", tag=f"s{half}", bufs=1)
            if half == 0:
                nc.tensor.matmul(
                    sp,
                    lhsT=phi[0:C8, mt * P : (mt + 1) * P],
                    rhs=theta[:, nsl],
                    start=True,
                    stop=True,
                )
            else:
                nc.tensor.matmul(
                    sp,
                    lhsT=phi[C8:P, mt * P : (mt + 1) * P],
                    rhs=TH2[i][C8:P, :],
                    start=True,
                    stop=True,
                    tile_position=(C8, 0),
                )
            et = etp.tile([P, SL], F32R, name="et", tag=f"et{mt}")
            nc.scalar.activation(et, sp, ACTF.Exp)
            ET[i][mt] = et

    # ---- phase 1: projections + pooling + g transposes -----------------
    with tc.tile_pool(name="ppsum", bufs=2, space="PSUM") as pp, tc.tile_pool(
        name="tpsum", bufs=1, space="PSUM"
    ) as tp:
        for ns in range(NS):
            nsl = slice(ns * SL, (ns + 1) * SL)
            msl = slice(ns * P, (ns + 1) * P)
            xr = []
            for k in range(4):
                t = stream.tile([P, SL], F32R, name="xr", tag=f"xr{k}")
                if k == 0:
                    nc.vector.tensor_copy(t, xf[k][:, nsl])
                else:
                    nc.scalar.copy(out=t, in_=xf[k][:, nsl])
                xr.append(t)
            ps = [
                pp.tile(
                    [P, SL], F32, name="pp", tag=f"pp{mt}",
                    bufs=(1 if mt == 2 else 2),
                )
                for mt in range(3)
            ]
            for mt in range(3):
                for k in range(4):
                    nc.tensor.matmul(
                        ps[mt],
                        lhsT=wp[k][:, mt * P : (mt + 1) * P],
                        rhs=xr[k],
                        start=(k == 0),
                        stop=(k == 3),
                    )
            nc.scalar.copy(out=theta[:, nsl], in_=ps[0][0:C8, :])
            nc.vector.tensor_reduce(
                out=phi[C8:P, msl],
                in_=_pool_view(ps[0][C8:P, :]),
                axis=AX.XY,
                op=ALU.max,
            )
            for i in range(2):
                nc.vector.tensor_reduce(
                    out=g[i][:, msl],
                    in_=_pool_view(ps[1 + i]),
                    axis=AX.XY,
                    op=ALU.max,
                )
            # transpose this slice's pooled g columns into gT[ns]
            for i in range(2):
                t = tp.tile([P, P], F32, name="tp", tag="tp")
                nc.tensor.transpose(t, g[i][:, msl], ident)
                nc.vector.tensor_copy(out=gT[ns][:, i * P : (i + 1) * P], in_=t)
            nc.sync.dma_start(out=phi[0:C8, msl], in_=phi[C8:P, msl])
            if ns == 0:
                emit_th2(0)
            if ns % 2 == 1:
                emit_score_pair(0, ns // 2)

    # ---- phase 2: softmax / attend / project ---------------------------
    with tc.tile_pool(name="qpsum", bufs=2, space="PSUM") as qp:
        def emit_scores(i):
            emit_th2(i)
            for j in range(MT // 2):
                emit_score_pair(i, j)

        def emit_attend(i, lo, w):
            # attend + project + residual for columns [i*SL+lo, i*SL+lo+w)
            nsl = slice(i * SL + lo, i * SL + lo + w)
            esl = slice(lo, lo + w)
            zp = qp.tile([P, w], F32, name="zp", tag="z")
            ap = [qp.tile([P, w], F32, name="ap", tag="a") for _ in range(2)]
            for mt in range(MT):
                st, sp_ = (mt == 0), (mt == MT - 1)
                if mt % 2 == 0:
                    # sum adjacent E tiles on DVE so Z needs half the PE passes
                    fsum = miscp.tile([P, w], F32R, name="fsum", tag="fsum")
                    nc.vector.tensor_add(
                        fsum, ET[i][mt][:, esl], ET[i][mt + 1][:, esl]
                    )
                    nc.tensor.matmul(
                        zp, lhsT=ones, rhs=fsum, start=st, stop=(mt == MT - 2),
                        skip_group_check=True,
                    )
                for ct in range(2):
                    nc.tensor.matmul(
                        ap[ct],
                        lhsT=gT[mt][:, ct * P : (ct + 1) * P],
                        rhs=ET[i][mt][:, esl],
                        start=st,
                        stop=sp_,
                        skip_group_check=True,
                    )
            rinv = miscp.tile([P, w], F32, name="rinv", tag="rinv")
            nc.vector.reciprocal(rinv, zp)
            att = []
            for ct in range(2):
                t = miscp.tile([P, w], F32R, name="att", tag=f"att{ct}")
                nc.vector.tensor_mul(t, ap[ct], rinv)
                att.append(t)
            for ot in range(4):
                op_ = qp.tile([P, w], F32, name="op", tag="o")
                for ct in range(2):
                    nc.tensor.matmul(
                        op_,
                        lhsT=wot[ct][:, ot * P : (ot + 1) * P],
                        rhs=att[ct],
                        start=(ct == 0),
                        stop=(ct == 1),
                    )
                ob = miscp.tile([P, w], F32, name="ob", tag=f"ob{ot % 2}")
                nc.vector.tensor_add(ob, op_, xf[ot][:, nsl])
                nc.sync.dma_start(out=out_d[ot * P : (ot + 1) * P, nsl], in_=ob)

        emit_scores(1)
        for i in range(NS):
            if i + 2 < NS:
                emit_scores(i + 2)
            emit_attend(i, 0, SL)


def build_nc():
    nc = bass.Bass(target_bir_lowering=False, trn_type="TRN2")
    with tile.TileContext(nc) as tc:
        with ExitStack() as ctx:
            emit(nc, tc, ctx)
    bass_rust.generate_event_semaphores(nc)
    return nc


def kernel(x, w_theta, w_phi, w_g, w_o, gamma):
    x = np.asarray(x, dtype=np.float32)
    B = x.shape[0]
    wproj = np.ascontiguousarray(
        np.concatenate(
            [np.asarray(w_theta).T, np.asarray(w_phi).T, np.asarray(w_g).T], axis=1
        ),
        dtype=np.float32,
    )
    wo_t = np.ascontiguousarray(
        (np.float32(gamma) * np.asarray(w_o)).T, dtype=np.float32
    )

    nc = build_nc()
    in_maps = []
    for b in range(B):
        xb = np.ascontiguousarray(x[b].reshape(C, N))
        in_maps.append({"x": xb, "wproj": wproj, "wo": wo_t})
    res = run_bass_kernel_spmd(nc, in_maps, core_ids=list(range(B)))
    out = np.stack(
        [res.results[b]["out"].reshape(C, 64, 64) for b in range(B)]
    ).astype(np.float32)
    return out


# revision 27
# speedup vs baseline: 1.3872x; 1.0179x over previous
"""SAGAN-style self-attention block on 8 trn2 NeuronCores.

Full inputs: x [8, 512, 64, 64], w_theta [64, 512], w_phi [64, 512],
w_g [256, 512], w_o [512, 256], gamma scalar.

Sharding: data-parallel over batch — one batch item per core. Each core runs
an identical Bass program over its own x[b]; weights are replicated.

Per-core math (C=512, n=H*W=4096, m=n/4=1024):
  theta = w_theta @ x            [64, 4096]
  phi   = pool2(w_phi @ x)       [64, 1024]
  g     = pool2(w_g @ x)         [256, 1024]
  S^T   = phi^T @ theta          [1024, 4096]   (scores, transposed layout)
  E     = exp(S^T)               (no max-subtraction needed: |S| < ~50)
  Z     = ones^T @ E             [*, 4096]      (row sums, broadcast layout)
  att   = (g @ E) / Z            [256, 4096]
  out   = (gamma*w_o) @ att + x  [512, 4096]

All matmuls run as float32r (full-rate fp32 on the PE at N>=512; tf32-like
input rounding, ~1.5e-4 rel err). The residual add uses unrounded fp32 x.
"""

from contextlib import ExitStack

import numpy as np

import bass_rust
import concourse.bass as bass
import concourse.mybir as mybir
import concourse.tile as tile
from concourse.bass_utils import run_bass_kernel_spmd
from concourse.masks import make_identity

P = 128
C = 512  # channels
C8 = 64  # theta/phi channels
C2 = 256  # g channels
N = 4096  # H*W
M = 1024  # pooled spatial
NS = 8  # n-slices
SL = 512  # n-slice width
MT = 8  # m-tiles of 128
F32 = mybir.dt.float32
F32R = mybir.dt.float32r
AX = mybir.AxisListType
ALU = mybir.AluOpType
ACTF = mybir.ActivationFunctionType


def _pool_view(ap):
    """[p, 512] slice of the conv output -> 5D maxpool view [p, h2, w2, dy, dx].

    Within an n-slice of 512 = 8 image rows: local n = (2*h2+dy)*64 + 2*w2+dx.
    """
    return ap.rearrange("p (h2 dy w2 dx) -> p h2 w2 dy dx", h2=4, dy=2, w2=32, dx=2)


def emit(nc, tc, ctx):
    x_f = nc.dram_tensor("x", [C, N], F32, kind="ExternalInput")
    wproj = nc.dram_tensor("wproj", [C, 384], F32R, kind="ExternalInput")
    wo = nc.dram_tensor("wo", [C2, C], F32R, kind="ExternalInput")
    out_d = nc.dram_tensor("out", [C, N], F32, kind="ExternalOutput")

    persist = ctx.enter_context(tc.tile_pool(name="persist", bufs=1))
    stream = ctx.enter_context(tc.tile_pool(name="stream", bufs=2))

    wpt = persist.tile([P, 4, 384], F32R, name="wpt")
    nc.scalar.dma_start(out=wpt, in_=wproj.ap().rearrange("(k p) o -> p k o", k=4))
    wp = [wpt[:, k, :] for k in range(4)]
    ones_f = persist.tile([P, P], F32)
    nc.vector.memset(ones_f, 1.0)
    ones = persist.tile([P, P], F32R)
    nc.vector.tensor_copy(ones, ones_f)
    ident = persist.tile([P, P], F32)
    make_identity(nc, ident)

    # x loads: slice-major chunks so phase-1 slice 0 unblocks after ~1MB
    xf = [persist.tile([P, N], F32, name=f"xf{cc}") for cc in range(4)]
    for q in range(NS):
        for cc in range(4):
            nc.sync.dma_start(
                out=xf[cc][:, q * SL : (q + 1) * SL],
                in_=x_f[cc * P : (cc + 1) * P, q * SL : (q + 1) * SL],
            )
    wot = []
    for k in range(2):
        t = persist.tile([P, C], F32R, name=f"wot{k}")
        nc.sync.dma_start(out=t, in_=wo[k * P : (k + 1) * P, :])
        wot.append(t)

    theta = persist.tile([C8, N], F32R)
    phi = persist.tile([P, M], F32R)  # [64:128] pooled, [0:64] copy for K rows 0-63
    g = [persist.tile([P, M], F32, name=f"g{i}") for i in range(2)]
    gT = [persist.tile([P, C2], F32R, name=f"gT{mt}") for mt in range(MT)]

    # score psum pool lives across phases 1+2 so slice-0 scores can start
    # inside phase 1
    spool = ctx.enter_context(tc.tile_pool(name="spsum", bufs=2, space="PSUM"))
    etp = ctx.enter_context(tc.tile_pool(name="et", bufs=2))
    miscp = ctx.enter_context(tc.tile_pool(name="misc", bufs=2))
    ET = [[None] * MT for _ in range(NS)]
    FS = [[None] * (MT // 2) for _ in range(NS)]
    TH2 = [None] * NS

    def emit_th2(i):
        nsl = slice(i * SL, (i + 1) * SL)
        t = miscp.tile([P, SL], F32R, name="th2", tag="th2", bufs=2)
        nc.sync.dma_start(out=t[C8:P, :], in_=theta[:, nsl])
        TH2[i] = t

    def emit_score_pair(i, j):
        # m-tiles 2j and 2j+1 run concurrently via row tiling (separate banks)
        nsl = slice(i * SL, (i + 1) * SL)
        for half, mt in enumerate((2 * j, 2 * j + 1)):
            sp = spool.tile([P, SL], F32, name="sp", tag=f"s{half}", bufs=1)
            if half == 0:
                nc.tensor.matmul(
                    sp,
                    lhsT=phi[0:C8, mt * P : (mt + 1) * P],
                    rhs=theta[:, nsl],
                    start=True,
                    stop=True,
                )
            else:
                nc.tensor.matmul(
                    sp,
                    lhsT=phi[C8:P, mt * P : (mt + 1) * P],
                    rhs=TH2[i][C8:P, :],
                    start=True,
                    stop=True,
                    tile_position=(C8, 0),
                )
            et = etp.tile([P, SL], F32R, name="et", tag=f"et{mt}")
            nc.scalar.activation(et, sp, ACTF.Exp)
            ET[i][mt] = et

    def emit_fsums(i):
        # pair-sums on DVE, one slice ahead of the attend stage's Z matmuls
        for j in range(MT // 2):
            fsum = miscp.tile([P, SL], F32R, name="fsum", tag=f"fsum{j}", bufs=2)
            nc.vector.tensor_add(fsum, ET[i][2 * j], ET[i][2 * j + 1])
            FS[i][j] = fsum

    # ---- phase 1: projections + pooling + g transposes -----------------
    with tc.tile_pool(name="ppsum", bufs=2, space="PSUM") as pp, tc.tile_pool(
        name="tpsum", bufs=1, space="PSUM"
    ) as tp:
        for ns in range(NS):
            nsl = slice(ns * SL, (ns + 1) * SL)
            msl = slice(ns * P, (ns + 1) * P)
            xr = []
            for k in range(4):
                t = stream.tile([P, SL], F32R, name="xr", tag=f"xr{k}")
                if k == 0:
                    nc.vector.tensor_copy(t, xf[k][:, nsl])
                else:
                    nc.scalar.copy(out=t, in_=xf[k][:, nsl])
                xr.append(t)
            ps = [
                pp.tile(
                    [P, SL], F32, name="pp", tag=f"pp{mt}",
                    bufs=(1 if mt == 2 else 2),
                )
                for mt in range(3)
            ]
            for mt in range(3):
                for k in range(4):
                    nc.tensor.matmul(
                        ps[mt],
                        lhsT=wp[k][:, mt * P : (mt + 1) * P],
                        rhs=xr[k],
                        start=(k == 0),
                        stop=(k == 3),
                    )
            nc.scalar.copy(out=theta[:, nsl], in_=ps[0][0:C8, :])
            nc.vector.tensor_reduce(
                out=phi[C8:P, msl],
                in_=_pool_view(ps[0][C8:P, :]),
                axis=AX.XY,
                op=ALU.max,
            )
            for i in range(2):
                nc.vector.tensor_reduce(
                    out=g[i][:, msl],
                    in_=_pool_view(ps[1 + i]),
                    axis=AX.XY,
                    op=ALU.max,
                )
            # transpose this slice's pooled g columns into gT[ns]
            for i in range(2):
                t = tp.tile([P, P], F32, name="tp", tag="tp")
                nc.tensor.transpose(t, g[i][:, msl], ident)
                nc.vector.tensor_copy(out=gT[ns][:, i * P : (i + 1) * P], in_=t)
            nc.sync.dma_start(out=phi[0:C8, msl], in_=phi[C8:P, msl])
            if ns == 0:
                emit_th2(0)
            if ns % 2 == 1:
                emit_score_pair(0, ns // 2)

    # ---- phase 2: softmax / attend / project ---------------------------
    with tc.tile_pool(name="qpsum", bufs=2, space="PSUM") as qp:
        def emit_scores(i):
            emit_th2(i)
            for j in range(MT // 2):
                emit_score_pair(i, j)

        def emit_attend(i, lo, w):
            # attend + project + residual for columns [i*SL+lo, i*SL+lo+w)
            nsl = slice(i * SL + lo, i * SL + lo + w)
            esl = slice(lo, lo + w)
            zp = qp.tile([P, w], F32, name="zp", tag="z")
            ap = [qp.tile([P, w], F32, name="ap", tag="a") for _ in range(2)]
            for mt in range(MT):
                st, sp_ = (mt == 0), (mt == MT - 1)
                if mt % 2 == 0:
                    nc.tensor.matmul(
                        zp,
                        lhsT=ones,
                        rhs=FS[i][mt // 2][:, esl],
                        start=st,
                        stop=(mt == MT - 2),
                        skip_group_check=True,
                    )
                for ct in range(2):
                    nc.tensor.matmul(
                        ap[ct],
                        lhsT=gT[mt][:, ct * P : (ct + 1) * P],
                        rhs=ET[i][mt][:, esl],
                        start=st,
                        stop=sp_,
                        skip_group_check=True,
                    )
            rinv = miscp.tile([P, w], F32, name="rinv", tag="rinv")
            nc.vector.reciprocal(rinv, zp)
            att = []
            for ct in range(2):
                t = miscp.tile([P, w], F32R, name="att", tag=f"att{ct}")
                nc.vector.tensor_mul(t, ap[ct], rinv)
                att.append(t)
            for ot in range(4):
                op_ = qp.tile([P, w], F32, name="op", tag="o")
                for ct in range(2):
                    nc.tensor.matmul(
                        op_,
                        lhsT=wot[ct][:, ot * P : (ot + 1) * P],
                        rhs=att[ct],
                        start=(ct == 0),
                        stop=(ct == 1),
                    )
                ob = miscp.tile([P, w], F32, name="ob", tag=f"ob{ot % 2}")
                nc.vector.tensor_add(ob, op_, xf[ot][:, nsl])
                nc.sync.dma_start(out=out_d[ot * P : (ot + 1) * P, nsl], in_=ob)

        emit_scores(1)
        emit_fsums(0)
        for i in range(NS):
            if i + 2 < NS:
                emit_scores(i + 2)
            if i + 1 < NS:
                emit_fsums(i + 1)
            emit_attend(i, 0, SL)


def build_nc():
    nc = bass.Bass(target_bir_lowering=False, trn_type="TRN2")
    with tile.TileContext(nc) as tc:
        with ExitStack() as ctx:
            emit(nc, tc, ctx)
    bass_rust.generate_event_semaphores(nc)
    return nc


def kernel(x, w_theta, w_phi, w_g, w_o, gamma):
    x = np.asarray(x, dtype=np.float32)
    B = x.shape[0]
    wproj = np.ascontiguousarray(
        np.concatenate(
            [np.asarray(w_theta).T, np.asarray(w_phi).T, np.asarray(w_g).T], axis=1
        ),
        dtype=np.float32,
    )
    wo_t = np.ascontiguousarray(
        (np.float32(gamma) * np.asarray(w_o)).T, dtype=np.float32
    )

    nc = build_nc()
    in_maps = []
    for b in range(B):
        xb = np.ascontiguousarray(x[b].reshape(C, N))
        in_maps.append({"x": xb, "wproj": wproj, "wo": wo_t})
    res = run_bass_kernel_spmd(nc, in_maps, core_ids=list(range(B)))
    out = np.stack(
        [res.results[b]["out"].reshape(C, 64, 64) for b in range(B)]
    ).astype(np.float32)
    return out
